# revision 20
# baseline (speedup 1.0000x reference)
"""MoE top-1 routing (ExpertAllocation) Trainium2 kernel.

Math: the reference's per-expert sort/cumsum/capacity-mask compares a cumsum of
*probabilities* (each <= 1) against a *count* capacity (T/E = 256).  Masking can
only trigger when some expert's total routed probability mass exceeds 256, i.e.
thousands of tokens routed to one expert.  The device kernel computes the
routed probs (softmax of the router logits) plus the per-expert routed mass
f_sum[e] (= sum of top-1 probs by argmax expert) and prob mass P_sum[e]; the
host checks max(f_sum) against the capacity and falls back to an exact numpy
implementation of the masking path if it could ever trigger (it does not for
any remotely balanced router).

Device layout per core (8-way token sharding, 2048 tokens/core):
  - input  xt  [4096, 2048] f32r : x-shard transposed on host (contraction dim
                                   on partitions -> contiguous DMAs)
  - input  w   [4096, 64]  f32r  : replicated router weight
  - input  bias[64, 1]      f32  : replicated router bias
  - output probs [2048, 64] f32  : softmax(x @ W + b) for this shard
  - output stats [64, 2]    f32  : col 0 = P_sum partial, col 1 = f_sum partial

Pipeline: k-chunk DMAs ([128, t_pass] slabs of xt) feed float32r matmuls with
the W chunk as the stationary operand, accumulating logits^T [64, 512] tiles in
PSUM across the 32 k-chunks.  Epilogue: ACT exp (bias fused), PE transpose back
to [token, expert], DVE row-reduces for Z/max, reciprocal, scale, is_equal
indicator, and two N=1 matmuls accumulating the per-expert stats in PSUM.

Constraint driving the structure: fused-weight-load matmuls (the only form for
4-byte dtypes) accept a single sync-wait, so every matmul may depend on at most
one semaphore.  All constants (W, bias, identity, ones) are therefore loaded in
a raw-bass preamble (manual DMA semaphore + all-engine barrier) before the
TileContext, making them dependency-free for the Tile scheduler; the PSUM->SBUF
copy after the transposes is pinned to the Scalar engine so a transpose's two
deps (exp producer, PSUM bank release) collapse onto one semaphore.
"""

import os
import sys
from contextlib import ExitStack

import numpy as np

for _p in ("/opt/trn_rl_repo", "/root/.axon_site/_ro/trn_rl_repo"):
    if os.path.isdir(_p) and _p not in sys.path:
        sys.path.append(_p)

import concourse.bass as bass
import concourse.bacc as bacc
import concourse.tile as tile
from concourse import mybir
from concourse.bass_utils import run_bass_kernel_spmd

N_CORES = 8
B, S, D, E = 4, 4096, 4096, 64
T = B * S
T_CORE = T // N_CORES
ALPHA = 0.01
CAPACITY_FACTOR = 1.0

F32 = mybir.dt.float32
F32R = mybir.dt.float32r
BF16 = mybir.dt.bfloat16


def build_nc(t_core=T_CORE, d=D, pass_sizes=(1536, 512), xt_bufs=22):
    """Build the per-core Bass program (SPMD: identical on all cores)."""
    assert sum(pass_sizes) == t_core
    assert all(s % 512 == 0 for s in pass_sizes)
    kch = d // 128                 # contraction chunks
    n_pass = len(pass_sizes)
    n_grp_total = t_core // 512

    nc = bacc.Bacc("TRN2", target_bir_lowering=False, num_devices=N_CORES)

    xt = nc.dram_tensor("xt", [d, t_core], F32R, kind="ExternalInput").ap()
    w = nc.dram_tensor("w", [128, kch * E], F32R, kind="ExternalInput").ap()
    bias = nc.dram_tensor("bias", [E, 1], F32, kind="ExternalInput").ap()
    ident_in = nc.dram_tensor("ident", [64, 64], F32, kind="ExternalInput").ap()
    ones_in = nc.dram_tensor("ones", [128, 1], F32, kind="ExternalInput").ap()
    probs_out = nc.dram_tensor("probs", [t_core, E], F32, kind="ExternalOutput").ap()
    stats_out = nc.dram_tensor(
        "stats", [n_grp_total, 2, E], F32, kind="ExternalOutput"
    ).ap()

    with ExitStack() as stack:
        # ---- raw-bass preamble: constants, dep-free for Tile ----
        pre_sem = stack.enter_context(nc.semaphore())
        w_sb = nc.alloc_sbuf_tensor("w_sb", [128, kch, E], F32R).ap()
        bias_sb = nc.alloc_sbuf_tensor("bias_sb", [E, 1], F32).ap()
        ident = nc.alloc_sbuf_tensor("ident_sb", [64, 64], F32).ap()
        ones_sb = nc.alloc_sbuf_tensor("ones_c_sb", [128, 1], F32).ap()
        nc.sync.dma_start(
            out=w_sb.rearrange("p k e -> p (k e)"), in_=w
        ).then_inc(pre_sem, 16)
        nc.sync.dma_start(out=bias_sb, in_=bias).then_inc(pre_sem, 16)
        nc.sync.dma_start(out=ident, in_=ident_in).then_inc(pre_sem, 16)
        nc.sync.dma_start(out=ones_sb, in_=ones_in).then_inc(pre_sem, 16)

        warm_sem = stack.enter_context(nc.semaphore())
        warm_a = nc.alloc_sbuf_tensor("warm_a", [128, 64], BF16).ap()
        warm_b = nc.alloc_sbuf_tensor("warm_b", [128, 512], BF16).ap()
        nc.vector.memset(warm_a, 1.0).then_inc(warm_sem, 1)
        nc.vector.memset(warm_b, 1.0).then_inc(warm_sem, 1)
        # HAM warm-up: fp32r matmuls never un-throttle the PE clock (they do
        # keep it warm once running); a short dense bf16 burst promotes the
        # clock to 2.4 GHz.  Runs concurrently with the preamble DMAs (own
        # semaphore) so it costs no wall-clock.
        nc.tensor.wait_ge(warm_sem, 2)
        with nc.psum_tensor([64, 512], F32) as warm_ps:
            # freed before TileContext; later PSUM reuse is safe because all
            # PSUM consumers sync on PE producers that follow these in order.
            for i in range(10):
                nc.tensor.matmul(
                    warm_ps.ap(), warm_a, warm_b, start=(i == 0), stop=(i == 9)
                )
        # Each compute engine waits for the preamble loads before its first
        # Tile instruction; the Sync/SP engine does NOT wait, so xt DMA issue
        # starts immediately (an all-engine barrier here would stall the DMA
        # pipeline behind the 1 MiB weight load).
        nc.tensor.wait_ge(pre_sem, 64)
        nc.scalar.wait_ge(pre_sem, 64)
        nc.vector.wait_ge(pre_sem, 64)
        nc.gpsimd.wait_ge(pre_sem, 64)

        with tile.TileContext(nc) as tc:
            with (
                tc.tile_pool(name="xtp", bufs=16) as xtp,
                tc.tile_pool(name="expp", bufs=2) as expp,
                tc.tile_pool(name="expTp", bufs=2) as expTp,
                tc.tile_pool(name="stagep", bufs=2) as stagep,
                tc.tile_pool(name="smallp", bufs=4) as smallp,
                tc.tile_pool(name="psL", bufs=n_grp_total, space="PSUM") as psL,
                tc.tile_pool(name="psT", bufs=2, space="PSUM") as psT,
                tc.tile_pool(name="psS", bufs=1, space="PSUM") as psS,
            ):
                t_off = 0
                grp_base = 0
                for p, t_pass in enumerate(pass_sizes):
                    n_grp = t_pass // 512
                    logits = [
                        psL.tile([E, 512], F32, tag="L", name=f"logits_{p}_{g}")
                        for g in range(n_grp)
                    ]
                    for k in range(kch):
                        xt_t = xtp.tile(
                            [128, t_pass], F32R, tag=f"xt{p}", name=f"xt_{p}_{k}"
                        )
                        nc.sync.dma_start(
                            out=xt_t,
                            in_=xt[k * 128 : (k + 1) * 128, t_off : t_off + t_pass],
                        )
                        for g in range(n_grp):
                            nc.tensor.matmul(
                                logits[g][:, :],
                                w_sb[:, k, :],
                                xt_t[:, g * 512 : (g + 1) * 512],
                                start=(k == 0),
                                stop=(k == kch - 1),
                            )

                    stage = stagep.tile(
                        [128, t_pass // 128, E], F32, tag="stage", name=f"stage{p}"
                    )
                    ind_stage = stagep.tile(
                        [128, t_pass // 128, E], F32, tag="ind_stage", name=f"ind{p}"
                    )
                    for g in range(n_grp):
                        gsl = slice(g * 4, (g + 1) * 4)
                        exp_sb = expp.tile([E, 512], F32, tag="exp", name=f"exp_{p}_{g}")
                        nc.scalar.activation(
                            out=exp_sb,
                            in_=logits[g][:, :],
                            func=mybir.ActivationFunctionType.Exp,
                            bias=bias_sb,
                            scale=1.0,
                        )
                        expT_ps = psT.tile([128, 4, E], F32, tag="expT_ps", name=f"expT_ps_{p}_{g}")
                        for j in range(4):
                            nc.tensor.transpose(
                                expT_ps[:, j, :],
                                exp_sb[:, j * 128 : (j + 1) * 128],
                                ident,
                            )
                        expT_sb = expTp.tile([128, 4, E], F32, tag="expT_sb", name=f"expT_sb_{p}_{g}")
                        nc.scalar.copy(expT_sb, expT_ps)

                        zz = smallp.tile([128, 4], F32, tag="z", name=f"z{p}{g}")
                        mx = smallp.tile([128, 4], F32, tag="m", name=f"m{p}{g}")
                        rz = smallp.tile([128, 4], F32, tag="rz", name=f"rz{p}{g}")
                        tp = smallp.tile([128, 4], F32, tag="tp", name=f"tp{p}{g}")
                        nc.vector.reduce_sum(zz, expT_sb, axis=mybir.AxisListType.X)
                        nc.vector.reduce_max(mx, expT_sb, axis=mybir.AxisListType.X)
                        nc.vector.reciprocal(rz, zz)
                        nc.vector.tensor_mul(tp, mx, rz)

                        nc.vector.tensor_tensor(
                            out=stage[:, gsl, :],
                            in0=expT_sb,
                            in1=rz.unsqueeze(-1).broadcast_to([128, 4, E]),
                            op=mybir.AluOpType.mult,
                        )
                        # weighted argmax indicator: (exp==max ? 1 : 0) * top_prob
                        nc.vector.tensor_tensor(
                            out=ind_stage[:, gsl, :],
                            in0=expT_sb,
                            in1=mx.unsqueeze(-1).broadcast_to([128, 4, E]),
                            op=mybir.AluOpType.is_equal,
                        )
                        nc.vector.tensor_tensor(
                            out=ind_stage[:, gsl, :],
                            in0=ind_stage[:, gsl, :],
                            in1=tp.unsqueeze(-1).broadcast_to([128, 4, E]),
                            op=mybir.AluOpType.mult,
                        )

                        # per-group stats matmuls: ones.T @ stage -> per-(block,e)
                        # column sums on partition 0, then reduce to per-expert
                        stats_ps = psS.tile([1, 4, E], F32, tag="sp", name=f"sp{p}{g}")
                        statf_ps = psS.tile([1, 4, E], F32, tag="sf", name=f"sf{p}{g}")
                        nc.tensor.matmul(
                            stats_ps[:, :, :],
                            ones_sb,
                            stage[:, gsl, :].rearrange("t i e -> t (i e)"),
                            start=True,
                            stop=True,
                        )
                        nc.tensor.matmul(
                            statf_ps[:, :, :],
                            ones_sb,
                            ind_stage[:, gsl, :].rearrange("t i e -> t (i e)"),
                            start=True,
                            stop=True,
                        )
                        stats_sb = smallp.tile(
                            [1, 2, E], F32, tag="statsb", name=f"ssb{p}{g}"
                        )
                        nc.vector.reduce_sum(
                            stats_sb[:, 0, :],
                            stats_ps.rearrange("t i e -> t e i"),
                            axis=mybir.AxisListType.X,
                        )
                        nc.vector.reduce_sum(
                            stats_sb[:, 1, :],
                            statf_ps.rearrange("t i e -> t e i"),
                            axis=mybir.AxisListType.X,
                        )
                        nc.gpsimd.dma_start(
                            out=stats_out[grp_base + g], in_=stats_sb
                        )
                        nc.gpsimd.dma_start(
                            out=probs_out[
                                t_off + g * 512 : t_off + (g + 1) * 512, :
                            ].rearrange("(i t) e -> t i e", t=128),
                            in_=stage[:, gsl, :],
                        )
                    t_off += t_pass
                    grp_base += n_grp

    nc.compile()
    return nc


_IDENT = np.eye(64, dtype=np.float32)
_ONES = np.ones((128, 1), dtype=np.float32)

_NC_CACHE = {}


def get_nc(**kwargs):
    key = tuple(sorted(kwargs.items()))
    if key not in _NC_CACHE:
        _NC_CACHE[key] = build_nc(**kwargs)
    return _NC_CACHE[key]


def _numpy_fallback(x, W, b):
    """Exact numpy mirror of the reference (only used if capacity masking
    could trigger, which requires a pathologically imbalanced router)."""
    Tt = x.shape[0] * x.shape[1]
    cap = int(Tt / E * CAPACITY_FACTOR)
    logits = x.reshape(-1, D) @ W + b
    m = logits.max(-1, keepdims=True)
    p = np.exp(logits - m)
    p = p / p.sum(-1, keepdims=True)
    ti = p.argmax(-1)
    tp = p.max(-1)
    flat = np.zeros_like(p)
    flat[np.arange(p.shape[0]), ti] = tp
    order = np.argsort(-flat, axis=0, kind="stable")
    sortedp = np.take_along_axis(flat, order, 0)
    cum = np.cumsum(sortedp, axis=0, dtype=np.float32)
    masked = np.where(cum <= cap, sortedp, np.float32(0.0))
    back = np.zeros_like(flat)
    np.put_along_axis(back, order, masked, 0)
    assigned = back.sum(-1) > 0
    out = p * assigned[:, None]
    f = flat.sum(0) / np.float32(Tt)
    P = p.sum(0) / np.float32(Tt)
    aux = np.float32(ALPHA * E) * np.float32((f * P).sum())
    return out.reshape(x.shape[0], x.shape[1], E).astype(np.float32), np.float32(aux)


def kernel(x, W, b, _trace=False, _tmpdir=None):
    x = np.ascontiguousarray(np.asarray(x, dtype=np.float32))
    W = np.ascontiguousarray(np.asarray(W, dtype=np.float32))
    b = np.ascontiguousarray(np.asarray(b, dtype=np.float32))

    nc = get_nc()
    kch = D // 128
    w_perm = np.ascontiguousarray(
        W.reshape(kch, 128, E).transpose(1, 0, 2).reshape(128, kch * E)
    )
    xflat = x.reshape(T, D)
    bcol = np.ascontiguousarray(b.reshape(E, 1))
    in_maps = []
    for c in range(N_CORES):
        shard = np.ascontiguousarray(xflat[c * T_CORE : (c + 1) * T_CORE].T)
        in_maps.append({"xt": shard, "w": w_perm, "bias": bcol, "ident": _IDENT, "ones": _ONES})

    res = run_bass_kernel_spmd(
        nc,
        in_maps,
        core_ids=list(range(N_CORES)),
        trace=_trace,
        tmpdir=_tmpdir,
    )

    probs = np.concatenate([r["probs"] for r in res.results], axis=0)
    stats = np.sum([r["stats"].sum(axis=0) for r in res.results], axis=0)
    p_sum, f_sum = stats[0], stats[1]

    cap = int(T / E * CAPACITY_FACTOR)
    if f_sum.max() > cap - 1.0:
        # Some expert's routed mass is near/over capacity: masking could
        # change the output.  Delegate to the exact host implementation.
        out, aux = _numpy_fallback(x, W, b)
        if _trace:
            return (out, aux), res
        return out, aux

    f = f_sum.astype(np.float32) / np.float32(T)
    P = p_sum.astype(np.float32) / np.float32(T)
    aux = np.float32(ALPHA * E) * np.float32((f * P).sum())
    out = probs.reshape(B, S, E)
    if _trace:
        return (out, aux), res
    return out, aux


# revision 22
# speedup vs baseline: 1.0041x; 1.0041x over previous
"""MoE top-1 routing (ExpertAllocation) Trainium2 kernel.

Math: the reference's per-expert sort/cumsum/capacity-mask compares a cumsum of
*probabilities* (each <= 1) against a *count* capacity (T/E = 256).  Masking can
only trigger when some expert's total routed probability mass exceeds 256, i.e.
thousands of tokens routed to one expert.  The device kernel computes the
routed probs (softmax of the router logits) plus the per-expert routed mass
f_sum[e] (= sum of top-1 probs by argmax expert) and prob mass P_sum[e]; the
host checks max(f_sum) against the capacity and falls back to an exact numpy
implementation of the masking path if it could ever trigger (it does not for
any remotely balanced router).

Device layout per core (8-way token sharding, 2048 tokens/core):
  - input  xt  [4096, 2048] f32r : x-shard transposed on host (contraction dim
                                   on partitions -> contiguous DMAs)
  - input  w   [4096, 64]  f32r  : replicated router weight
  - input  bias[64, 1]      f32  : replicated router bias
  - output probs [2048, 64] f32  : softmax(x @ W + b) for this shard
  - output stats [64, 2]    f32  : col 0 = P_sum partial, col 1 = f_sum partial

Pipeline: k-chunk DMAs ([128, t_pass] slabs of xt) feed float32r matmuls with
the W chunk as the stationary operand, accumulating logits^T [64, 512] tiles in
PSUM across the 32 k-chunks.  Epilogue: ACT exp (bias fused), PE transpose back
to [token, expert], DVE row-reduces for Z/max, reciprocal, scale, is_equal
indicator, and two N=1 matmuls accumulating the per-expert stats in PSUM.

Constraint driving the structure: fused-weight-load matmuls (the only form for
4-byte dtypes) accept a single sync-wait, so every matmul may depend on at most
one semaphore.  All constants (W, bias, identity, ones) are therefore loaded in
a raw-bass preamble (manual DMA semaphore + all-engine barrier) before the
TileContext, making them dependency-free for the Tile scheduler; the PSUM->SBUF
copy after the transposes is pinned to the Scalar engine so a transpose's two
deps (exp producer, PSUM bank release) collapse onto one semaphore.
"""

import os
import sys
from contextlib import ExitStack

import numpy as np

for _p in ("/opt/trn_rl_repo", "/root/.axon_site/_ro/trn_rl_repo"):
    if os.path.isdir(_p) and _p not in sys.path:
        sys.path.append(_p)

import concourse.bass as bass
import concourse.bacc as bacc
import concourse.tile as tile
from concourse import mybir
from concourse.bass_utils import run_bass_kernel_spmd

N_CORES = 8
B, S, D, E = 4, 4096, 4096, 64
T = B * S
T_CORE = T // N_CORES
ALPHA = 0.01
CAPACITY_FACTOR = 1.0

F32 = mybir.dt.float32
F32R = mybir.dt.float32r
BF16 = mybir.dt.bfloat16


def build_nc(t_core=T_CORE, d=D, pass_sizes=(1024, 1024), xt_bufs=22):
    """Build the per-core Bass program (SPMD: identical on all cores)."""
    assert sum(pass_sizes) == t_core
    assert all(s % 512 == 0 for s in pass_sizes)
    kch = d // 128                 # contraction chunks
    n_pass = len(pass_sizes)
    n_grp_total = t_core // 512

    nc = bacc.Bacc("TRN2", target_bir_lowering=False, num_devices=N_CORES)

    xt = nc.dram_tensor("xt", [d, t_core], F32R, kind="ExternalInput").ap()
    w = nc.dram_tensor("w", [128, kch * E], F32R, kind="ExternalInput").ap()
    bias = nc.dram_tensor("bias", [E, 1], F32, kind="ExternalInput").ap()
    ident_in = nc.dram_tensor("ident", [64, 64], F32, kind="ExternalInput").ap()
    ones_in = nc.dram_tensor("ones", [128, 1], F32, kind="ExternalInput").ap()
    probs_out = nc.dram_tensor("probs", [t_core, E], F32, kind="ExternalOutput").ap()
    stats_out = nc.dram_tensor(
        "stats", [n_grp_total, 2, E], F32, kind="ExternalOutput"
    ).ap()

    with ExitStack() as stack:
        # ---- raw-bass preamble: constants, dep-free for Tile ----
        pre_sem = stack.enter_context(nc.semaphore())
        w_sb = nc.alloc_sbuf_tensor("w_sb", [128, kch, E], F32R).ap()
        bias_sb = nc.alloc_sbuf_tensor("bias_sb", [E, 1], F32).ap()
        ident = nc.alloc_sbuf_tensor("ident_sb", [64, 64], F32).ap()
        ones_sb = nc.alloc_sbuf_tensor("ones_c_sb", [128, 1], F32).ap()
        nc.sync.dma_start(
            out=w_sb.rearrange("p k e -> p (k e)"), in_=w
        ).then_inc(pre_sem, 16)
        nc.sync.dma_start(out=bias_sb, in_=bias).then_inc(pre_sem, 16)
        nc.sync.dma_start(out=ident, in_=ident_in).then_inc(pre_sem, 16)
        nc.sync.dma_start(out=ones_sb, in_=ones_in).then_inc(pre_sem, 16)

        warm_sem = stack.enter_context(nc.semaphore())
        warm_a = nc.alloc_sbuf_tensor("warm_a", [128, 64], BF16).ap()
        warm_b = nc.alloc_sbuf_tensor("warm_b", [128, 512], BF16).ap()
        nc.vector.memset(warm_a, 1.0).then_inc(warm_sem, 1)
        nc.vector.memset(warm_b, 1.0).then_inc(warm_sem, 1)
        # HAM warm-up: fp32r matmuls never un-throttle the PE clock (they do
        # keep it warm once running); a short dense bf16 burst promotes the
        # clock to 2.4 GHz.  Runs concurrently with the preamble DMAs (own
        # semaphore) so it costs no wall-clock.
        nc.tensor.wait_ge(warm_sem, 2)
        with nc.psum_tensor([64, 512], F32) as warm_ps:
            # freed before TileContext; later PSUM reuse is safe because all
            # PSUM consumers sync on PE producers that follow these in order.
            for i in range(10):
                nc.tensor.matmul(
                    warm_ps.ap(), warm_a, warm_b, start=(i == 0), stop=(i == 9)
                )
        # Each compute engine waits for the preamble loads before its first
        # Tile instruction; the Sync/SP engine does NOT wait, so xt DMA issue
        # starts immediately (an all-engine barrier here would stall the DMA
        # pipeline behind the 1 MiB weight load).
        nc.tensor.wait_ge(pre_sem, 64)
        nc.scalar.wait_ge(pre_sem, 64)
        nc.vector.wait_ge(pre_sem, 64)
        nc.gpsimd.wait_ge(pre_sem, 64)

        with tile.TileContext(nc) as tc:
            with (
                tc.tile_pool(name="xtp", bufs=xt_bufs) as xtp,
                tc.tile_pool(name="expp", bufs=2) as expp,
                tc.tile_pool(name="expTp", bufs=2) as expTp,
                tc.tile_pool(name="stagep", bufs=2) as stagep,
                tc.tile_pool(name="smallp", bufs=4) as smallp,
                tc.tile_pool(name="psL", bufs=n_grp_total, space="PSUM") as psL,
                tc.tile_pool(name="psT", bufs=2, space="PSUM") as psT,
                tc.tile_pool(name="psS", bufs=1, space="PSUM") as psS,
            ):
                t_off = 0
                grp_base = 0
                for p, t_pass in enumerate(pass_sizes):
                    n_grp = t_pass // 512
                    logits = [
                        psL.tile([E, 512], F32, tag="L", name=f"logits_{p}_{g}")
                        for g in range(n_grp)
                    ]
                    for k in range(kch):
                        xt_t = xtp.tile(
                            [128, t_pass], F32R, tag="xt", name=f"xt_{p}_{k}"
                        )
                        nc.sync.dma_start(
                            out=xt_t,
                            in_=xt[k * 128 : (k + 1) * 128, t_off : t_off + t_pass],
                        )
                        for g in range(n_grp):
                            nc.tensor.matmul(
                                logits[g][:, :],
                                w_sb[:, k, :],
                                xt_t[:, g * 512 : (g + 1) * 512],
                                start=(k == 0),
                                stop=(k == kch - 1),
                            )

                    stage = stagep.tile(
                        [128, t_pass // 128, E], F32, tag="stage", name=f"stage{p}"
                    )
                    ind_stage = stagep.tile(
                        [128, t_pass // 128, E], F32, tag="ind_stage", name=f"ind{p}"
                    )
                    for g in range(n_grp):
                        gsl = slice(g * 4, (g + 1) * 4)
                        exp_sb = expp.tile([E, 512], F32, tag="exp", name=f"exp_{p}_{g}")
                        nc.scalar.activation(
                            out=exp_sb,
                            in_=logits[g][:, :],
                            func=mybir.ActivationFunctionType.Exp,
                            bias=bias_sb,
                            scale=1.0,
                        )
                        expT_ps = psT.tile([128, 4, E], F32, tag="expT_ps", name=f"expT_ps_{p}_{g}")
                        for j in range(4):
                            nc.tensor.transpose(
                                expT_ps[:, j, :],
                                exp_sb[:, j * 128 : (j + 1) * 128],
                                ident,
                            )
                        expT_sb = expTp.tile([128, 4, E], F32, tag="expT_sb", name=f"expT_sb_{p}_{g}")
                        nc.scalar.copy(expT_sb, expT_ps)

                        zz = smallp.tile([128, 4], F32, tag="z", name=f"z{p}{g}")
                        mx = smallp.tile([128, 4], F32, tag="m", name=f"m{p}{g}")
                        rz = smallp.tile([128, 4], F32, tag="rz", name=f"rz{p}{g}")
                        tp = smallp.tile([128, 4], F32, tag="tp", name=f"tp{p}{g}")
                        nc.vector.reduce_sum(zz, expT_sb, axis=mybir.AxisListType.X)
                        nc.vector.reduce_max(mx, expT_sb, axis=mybir.AxisListType.X)
                        nc.vector.reciprocal(rz, zz)
                        nc.vector.tensor_mul(tp, mx, rz)

                        nc.vector.tensor_tensor(
                            out=stage[:, gsl, :],
                            in0=expT_sb,
                            in1=rz.unsqueeze(-1).broadcast_to([128, 4, E]),
                            op=mybir.AluOpType.mult,
                        )
                        # weighted argmax indicator: (exp==max ? 1 : 0) * top_prob
                        nc.vector.tensor_tensor(
                            out=ind_stage[:, gsl, :],
                            in0=expT_sb,
                            in1=mx.unsqueeze(-1).broadcast_to([128, 4, E]),
                            op=mybir.AluOpType.is_equal,
                        )
                        nc.vector.tensor_tensor(
                            out=ind_stage[:, gsl, :],
                            in0=ind_stage[:, gsl, :],
                            in1=tp.unsqueeze(-1).broadcast_to([128, 4, E]),
                            op=mybir.AluOpType.mult,
                        )

                        # per-group stats matmuls: ones.T @ stage -> per-(block,e)
                        # column sums on partition 0, then reduce to per-expert
                        stats_ps = psS.tile(
                            [1, 8, E], F32, tag="sp", bufs=2, name=f"sp{p}{g}"
                        )
                        nc.tensor.matmul(
                            stats_ps[:, 0:4, :],
                            ones_sb,
                            stage[:, gsl, :].rearrange("t i e -> t (i e)"),
                            start=True,
                            stop=True,
                        )
                        nc.tensor.matmul(
                            stats_ps[:, 4:8, :],
                            ones_sb,
                            ind_stage[:, gsl, :].rearrange("t i e -> t (i e)"),
                            start=True,
                            stop=True,
                        )
                        stats_sb = smallp.tile(
                            [1, 2, E], F32, tag="statsb", name=f"ssb{p}{g}"
                        )
                        nc.vector.reduce_sum(
                            stats_sb[:, 0, :],
                            stats_ps[:, 0:4, :].rearrange("t i e -> t e i"),
                            axis=mybir.AxisListType.X,
                        )
                        nc.vector.reduce_sum(
                            stats_sb[:, 1, :],
                            stats_ps[:, 4:8, :].rearrange("t i e -> t e i"),
                            axis=mybir.AxisListType.X,
                        )
                        nc.gpsimd.dma_start(
                            out=stats_out[grp_base + g], in_=stats_sb
                        )
                        nc.gpsimd.dma_start(
                            out=probs_out[
                                t_off + g * 512 : t_off + (g + 1) * 512, :
                            ].rearrange("(i t) e -> t i e", t=128),
                            in_=stage[:, gsl, :],
                        )
                    t_off += t_pass
                    grp_base += n_grp

    nc.compile()
    return nc


_IDENT = np.eye(64, dtype=np.float32)
_ONES = np.ones((128, 1), dtype=np.float32)

_NC_CACHE = {}


def get_nc(**kwargs):
    key = tuple(sorted(kwargs.items()))
    if key not in _NC_CACHE:
        _NC_CACHE[key] = build_nc(**kwargs)
    return _NC_CACHE[key]


def _numpy_fallback(x, W, b):
    """Exact numpy mirror of the reference (only used if capacity masking
    could trigger, which requires a pathologically imbalanced router)."""
    Tt = x.shape[0] * x.shape[1]
    cap = int(Tt / E * CAPACITY_FACTOR)
    logits = x.reshape(-1, D) @ W + b
    m = logits.max(-1, keepdims=True)
    p = np.exp(logits - m)
    p = p / p.sum(-1, keepdims=True)
    ti = p.argmax(-1)
    tp = p.max(-1)
    flat = np.zeros_like(p)
    flat[np.arange(p.shape[0]), ti] = tp
    order = np.argsort(-flat, axis=0, kind="stable")
    sortedp = np.take_along_axis(flat, order, 0)
    cum = np.cumsum(sortedp, axis=0, dtype=np.float32)
    masked = np.where(cum <= cap, sortedp, np.float32(0.0))
    back = np.zeros_like(flat)
    np.put_along_axis(back, order, masked, 0)
    assigned = back.sum(-1) > 0
    out = p * assigned[:, None]
    f = flat.sum(0) / np.float32(Tt)
    P = p.sum(0) / np.float32(Tt)
    aux = np.float32(ALPHA * E) * np.float32((f * P).sum())
    return out.reshape(x.shape[0], x.shape[1], E).astype(np.float32), np.float32(aux)


def kernel(x, W, b, _trace=False, _tmpdir=None):
    x = np.ascontiguousarray(np.asarray(x, dtype=np.float32))
    W = np.ascontiguousarray(np.asarray(W, dtype=np.float32))
    b = np.ascontiguousarray(np.asarray(b, dtype=np.float32))

    nc = get_nc()
    kch = D // 128
    w_perm = np.ascontiguousarray(
        W.reshape(kch, 128, E).transpose(1, 0, 2).reshape(128, kch * E)
    )
    xflat = x.reshape(T, D)
    bcol = np.ascontiguousarray(b.reshape(E, 1))
    in_maps = []
    for c in range(N_CORES):
        shard = np.ascontiguousarray(xflat[c * T_CORE : (c + 1) * T_CORE].T)
        in_maps.append({"xt": shard, "w": w_perm, "bias": bcol, "ident": _IDENT, "ones": _ONES})

    res = run_bass_kernel_spmd(
        nc,
        in_maps,
        core_ids=list(range(N_CORES)),
        trace=_trace,
        tmpdir=_tmpdir,
    )

    probs = np.concatenate([r["probs"] for r in res.results], axis=0)
    stats = np.sum([r["stats"].sum(axis=0) for r in res.results], axis=0)
    p_sum, f_sum = stats[0], stats[1]

    cap = int(T / E * CAPACITY_FACTOR)
    if f_sum.max() > cap - 1.0:
        # Some expert's routed mass is near/over capacity: masking could
        # change the output.  Delegate to the exact host implementation.
        out, aux = _numpy_fallback(x, W, b)
        if _trace:
            return (out, aux), res
        return out, aux

    f = f_sum.astype(np.float32) / np.float32(T)
    P = p_sum.astype(np.float32) / np.float32(T)
    aux = np.float32(ALPHA * E) * np.float32((f * P).sum())
    out = probs.reshape(B, S, E)
    if _trace:
        return (out, aux), res
    return out, aux


# revision 23
# speedup vs baseline: 1.1480x; 1.1433x over previous
"""MoE top-1 routing (ExpertAllocation) Trainium2 kernel.

Math: the reference's per-expert sort/cumsum/capacity-mask compares a cumsum of
*probabilities* (each <= 1) against a *count* capacity (T/E = 256).  Masking can
only trigger when some expert's total routed probability mass exceeds 256, i.e.
thousands of tokens routed to one expert.  The device kernel computes the
routed probs (softmax of the router logits) plus the per-expert routed mass
f_sum[e] (= sum of top-1 probs by argmax expert) and prob mass P_sum[e]; the
host checks max(f_sum) against the capacity and falls back to an exact numpy
implementation of the masking path if it could ever trigger (it does not for
any remotely balanced router).

Device layout per core (8-way token sharding, 2048 tokens/core):
  - input  xt  [4096, 2048] f32r : x-shard transposed on host (contraction dim
                                   on partitions -> contiguous DMAs)
  - input  w   [4096, 64]  f32r  : replicated router weight
  - input  bias[64, 1]      f32  : replicated router bias
  - output probs [2048, 64] f32  : softmax(x @ W + b) for this shard
  - output stats [64, 2]    f32  : col 0 = P_sum partial, col 1 = f_sum partial

Pipeline: k-chunk DMAs ([128, t_pass] slabs of xt) feed float32r matmuls with
the W chunk as the stationary operand, accumulating logits^T [64, 512] tiles in
PSUM across the 32 k-chunks.  Epilogue: ACT exp (bias fused), PE transpose back
to [token, expert], DVE row-reduces for Z/max, reciprocal, scale, is_equal
indicator, and two N=1 matmuls accumulating the per-expert stats in PSUM.

Constraint driving the structure: fused-weight-load matmuls (the only form for
4-byte dtypes) accept a single sync-wait, so every matmul may depend on at most
one semaphore.  All constants (W, bias, identity, ones) are therefore loaded in
a raw-bass preamble (manual DMA semaphore + all-engine barrier) before the
TileContext, making them dependency-free for the Tile scheduler; the PSUM->SBUF
copy after the transposes is pinned to the Scalar engine so a transpose's two
deps (exp producer, PSUM bank release) collapse onto one semaphore.
"""

import os
import sys
from contextlib import ExitStack

import numpy as np

for _p in ("/opt/trn_rl_repo", "/root/.axon_site/_ro/trn_rl_repo"):
    if os.path.isdir(_p) and _p not in sys.path:
        sys.path.append(_p)

import concourse.bass as bass
import concourse.bacc as bacc
import concourse.tile as tile
from concourse import mybir
from concourse.bass_utils import run_bass_kernel_spmd

N_CORES = 8
B, S, D, E = 4, 4096, 4096, 64
T = B * S
T_CORE = T // N_CORES
ALPHA = 0.01
CAPACITY_FACTOR = 1.0

F32 = mybir.dt.float32
F32R = mybir.dt.float32r
BF16 = mybir.dt.bfloat16


def build_nc(t_core=T_CORE, d=D, pass_sizes=(1024, 1024), xt_bufs=22):
    """Build the per-core Bass program (SPMD: identical on all cores)."""
    assert sum(pass_sizes) == t_core
    assert all(s % 512 == 0 for s in pass_sizes)
    kch = d // 128                 # contraction chunks
    n_pass = len(pass_sizes)
    n_grp_total = t_core // 512

    nc = bacc.Bacc("TRN2", target_bir_lowering=False, num_devices=N_CORES)

    xt = nc.dram_tensor("xt", [d, t_core], F32R, kind="ExternalInput").ap()
    w = nc.dram_tensor("w", [128, kch * E], F32R, kind="ExternalInput").ap()
    bias = nc.dram_tensor("bias", [E, 1], F32, kind="ExternalInput").ap()
    ident_in = nc.dram_tensor("ident", [64, 64], F32, kind="ExternalInput").ap()
    ones_in = nc.dram_tensor("ones", [128, 1], F32, kind="ExternalInput").ap()
    probs_out = nc.dram_tensor("probs", [t_core, E], F32, kind="ExternalOutput").ap()
    stats_out = nc.dram_tensor(
        "stats", [n_grp_total, 8, E], F32, kind="ExternalOutput"
    ).ap()

    with ExitStack() as stack:
        # ---- raw-bass preamble: constants, dep-free for Tile ----
        pre_sem = stack.enter_context(nc.semaphore())
        w_sb = nc.alloc_sbuf_tensor("w_sb", [128, kch, E], F32R).ap()
        bias_sb = nc.alloc_sbuf_tensor("bias_sb", [E, 1], F32).ap()
        ident = nc.alloc_sbuf_tensor("ident_sb", [64, 64], F32).ap()
        ones_sb = nc.alloc_sbuf_tensor("ones_c_sb", [128, 1], F32).ap()
        nc.sync.dma_start(
            out=w_sb.rearrange("p k e -> p (k e)"), in_=w
        ).then_inc(pre_sem, 16)
        nc.sync.dma_start(out=bias_sb, in_=bias).then_inc(pre_sem, 16)
        nc.sync.dma_start(out=ident, in_=ident_in).then_inc(pre_sem, 16)
        nc.sync.dma_start(out=ones_sb, in_=ones_in).then_inc(pre_sem, 16)

        warm_sem = stack.enter_context(nc.semaphore())
        warm_a = nc.alloc_sbuf_tensor("warm_a", [128, 64], BF16).ap()
        warm_b = nc.alloc_sbuf_tensor("warm_b", [128, 512], BF16).ap()
        nc.vector.memset(warm_a, 1.0).then_inc(warm_sem, 1)
        nc.vector.memset(warm_b, 1.0).then_inc(warm_sem, 1)
        # HAM warm-up: fp32r matmuls never un-throttle the PE clock (they do
        # keep it warm once running); a short dense bf16 burst promotes the
        # clock to 2.4 GHz.  Runs concurrently with the preamble DMAs (own
        # semaphore) so it costs no wall-clock.
        nc.tensor.wait_ge(warm_sem, 2)
        with nc.psum_tensor([64, 512], F32) as warm_ps:
            # freed before TileContext; later PSUM reuse is safe because all
            # PSUM consumers sync on PE producers that follow these in order.
            for i in range(10):
                nc.tensor.matmul(
                    warm_ps.ap(), warm_a, warm_b, start=(i == 0), stop=(i == 9)
                )
        # Each compute engine waits for the preamble loads before its first
        # Tile instruction; the Sync/SP engine does NOT wait, so xt DMA issue
        # starts immediately (an all-engine barrier here would stall the DMA
        # pipeline behind the 1 MiB weight load).
        nc.tensor.wait_ge(pre_sem, 64)
        nc.scalar.wait_ge(pre_sem, 64)
        nc.vector.wait_ge(pre_sem, 64)
        nc.gpsimd.wait_ge(pre_sem, 64)

        with tile.TileContext(nc) as tc:
            with (
                tc.tile_pool(name="xtp", bufs=xt_bufs) as xtp,
                tc.tile_pool(name="expp", bufs=2) as expp,
                tc.tile_pool(name="expTp", bufs=2) as expTp,
                tc.tile_pool(name="stagep", bufs=2) as stagep,
                tc.tile_pool(name="smallp", bufs=4) as smallp,
                tc.tile_pool(name="psL", bufs=n_grp_total, space="PSUM") as psL,
                tc.tile_pool(name="psT", bufs=2, space="PSUM") as psT,
                tc.tile_pool(name="psS", bufs=1, space="PSUM") as psS,
            ):
                t_off = 0
                grp_base = 0
                for p, t_pass in enumerate(pass_sizes):
                    n_grp = t_pass // 512
                    logits = [
                        psL.tile([E, 512], F32, tag="L", name=f"logits_{p}_{g}")
                        for g in range(n_grp)
                    ]
                    for k in range(kch):
                        xt_t = xtp.tile(
                            [128, t_pass], F32R, tag="xt", name=f"xt_{p}_{k}"
                        )
                        nc.sync.dma_start(
                            out=xt_t,
                            in_=xt[k * 128 : (k + 1) * 128, t_off : t_off + t_pass],
                        )
                        for g in range(n_grp):
                            nc.tensor.matmul(
                                logits[g][:, :],
                                w_sb[:, k, :],
                                xt_t[:, g * 512 : (g + 1) * 512],
                                start=(k == 0),
                                stop=(k == kch - 1),
                            )

                    stage = stagep.tile(
                        [128, t_pass // 128, E], F32, tag="stage", name=f"stage{p}"
                    )
                    ind_stage = stagep.tile(
                        [128, t_pass // 128, E], F32, tag="ind_stage", name=f"ind{p}"
                    )
                    for g in range(n_grp):
                        gsl = slice(g * 4, (g + 1) * 4)
                        exp_sb = expp.tile([E, 512], F32, tag="exp", name=f"exp_{p}_{g}")
                        nc.scalar.activation(
                            out=exp_sb,
                            in_=logits[g][:, :],
                            func=mybir.ActivationFunctionType.Exp,
                            bias=bias_sb,
                            scale=1.0,
                        )
                        expT_ps = psT.tile([128, 4, E], F32, tag="expT_ps", name=f"expT_ps_{p}_{g}")
                        for j in range(4):
                            nc.tensor.transpose(
                                expT_ps[:, j, :],
                                exp_sb[:, j * 128 : (j + 1) * 128],
                                ident,
                            )
                        expT_sb = expTp.tile([128, 4, E], F32, tag="expT_sb", name=f"expT_sb_{p}_{g}")
                        nc.scalar.copy(expT_sb, expT_ps)

                        zz = smallp.tile([128, 4], F32, tag="z", name=f"z{p}{g}")
                        mx = smallp.tile([128, 4], F32, tag="m", name=f"m{p}{g}")
                        rz = smallp.tile([128, 4], F32, tag="rz", name=f"rz{p}{g}")
                        tp = smallp.tile([128, 4], F32, tag="tp", name=f"tp{p}{g}")
                        nc.vector.reduce_sum(zz, expT_sb, axis=mybir.AxisListType.X)
                        nc.vector.reduce_max(mx, expT_sb, axis=mybir.AxisListType.X)
                        nc.vector.reciprocal(rz, zz)
                        nc.vector.tensor_mul(tp, mx, rz)

                        nc.vector.tensor_tensor(
                            out=stage[:, gsl, :],
                            in0=expT_sb,
                            in1=rz.unsqueeze(-1).broadcast_to([128, 4, E]),
                            op=mybir.AluOpType.mult,
                        )
                        # weighted argmax indicator: (exp==max ? 1 : 0) * top_prob
                        nc.vector.tensor_tensor(
                            out=ind_stage[:, gsl, :],
                            in0=expT_sb,
                            in1=mx.unsqueeze(-1).broadcast_to([128, 4, E]),
                            op=mybir.AluOpType.is_equal,
                        )
                        nc.vector.tensor_tensor(
                            out=ind_stage[:, gsl, :],
                            in0=ind_stage[:, gsl, :],
                            in1=tp.unsqueeze(-1).broadcast_to([128, 4, E]),
                            op=mybir.AluOpType.mult,
                        )

                        # per-group stats matmuls: ones.T @ stage -> per-(block,e)
                        # column sums on partition 0, then reduce to per-expert
                        stats_ps = psS.tile(
                            [1, 8, E], F32, tag="sp", bufs=2, name=f"sp{p}{g}"
                        )
                        nc.tensor.matmul(
                            stats_ps[:, 0:4, :],
                            ones_sb,
                            stage[:, gsl, :].rearrange("t i e -> t (i e)"),
                            start=True,
                            stop=True,
                        )
                        nc.tensor.matmul(
                            stats_ps[:, 4:8, :],
                            ones_sb,
                            ind_stage[:, gsl, :].rearrange("t i e -> t (i e)"),
                            start=True,
                            stop=True,
                        )
                        stats_sb = smallp.tile(
                            [1, 8, E], F32, tag="statsb", name=f"ssb{p}{g}"
                        )
                        nc.scalar.copy(stats_sb, stats_ps)
                        nc.gpsimd.dma_start(
                            out=stats_out[grp_base + g], in_=stats_sb
                        )
                        nc.gpsimd.dma_start(
                            out=probs_out[
                                t_off + g * 512 : t_off + (g + 1) * 512, :
                            ].rearrange("(i t) e -> t i e", t=128),
                            in_=stage[:, gsl, :],
                        )
                    t_off += t_pass
                    grp_base += n_grp

    nc.compile()
    return nc


_IDENT = np.eye(64, dtype=np.float32)
_ONES = np.ones((128, 1), dtype=np.float32)

_NC_CACHE = {}


def get_nc(**kwargs):
    key = tuple(sorted(kwargs.items()))
    if key not in _NC_CACHE:
        _NC_CACHE[key] = build_nc(**kwargs)
    return _NC_CACHE[key]


def _numpy_fallback(x, W, b):
    """Exact numpy mirror of the reference (only used if capacity masking
    could trigger, which requires a pathologically imbalanced router)."""
    Tt = x.shape[0] * x.shape[1]
    cap = int(Tt / E * CAPACITY_FACTOR)
    logits = x.reshape(-1, D) @ W + b
    m = logits.max(-1, keepdims=True)
    p = np.exp(logits - m)
    p = p / p.sum(-1, keepdims=True)
    ti = p.argmax(-1)
    tp = p.max(-1)
    flat = np.zeros_like(p)
    flat[np.arange(p.shape[0]), ti] = tp
    order = np.argsort(-flat, axis=0, kind="stable")
    sortedp = np.take_along_axis(flat, order, 0)
    cum = np.cumsum(sortedp, axis=0, dtype=np.float32)
    masked = np.where(cum <= cap, sortedp, np.float32(0.0))
    back = np.zeros_like(flat)
    np.put_along_axis(back, order, masked, 0)
    assigned = back.sum(-1) > 0
    out = p * assigned[:, None]
    f = flat.sum(0) / np.float32(Tt)
    P = p.sum(0) / np.float32(Tt)
    aux = np.float32(ALPHA * E) * np.float32((f * P).sum())
    return out.reshape(x.shape[0], x.shape[1], E).astype(np.float32), np.float32(aux)


def kernel(x, W, b, _trace=False, _tmpdir=None):
    x = np.ascontiguousarray(np.asarray(x, dtype=np.float32))
    W = np.ascontiguousarray(np.asarray(W, dtype=np.float32))
    b = np.ascontiguousarray(np.asarray(b, dtype=np.float32))

    nc = get_nc()
    kch = D // 128
    w_perm = np.ascontiguousarray(
        W.reshape(kch, 128, E).transpose(1, 0, 2).reshape(128, kch * E)
    )
    xflat = x.reshape(T, D)
    bcol = np.ascontiguousarray(b.reshape(E, 1))
    in_maps = []
    for c in range(N_CORES):
        shard = np.ascontiguousarray(xflat[c * T_CORE : (c + 1) * T_CORE].T)
        in_maps.append({"xt": shard, "w": w_perm, "bias": bcol, "ident": _IDENT, "ones": _ONES})

    res = run_bass_kernel_spmd(
        nc,
        in_maps,
        core_ids=list(range(N_CORES)),
        trace=_trace,
        tmpdir=_tmpdir,
    )

    probs = np.concatenate([r["probs"] for r in res.results], axis=0)
    stats = np.sum([r["stats"] for r in res.results], axis=0)  # [n_grp, 8, E]
    p_sum = stats[:, 0:4, :].sum(axis=(0, 1))
    f_sum = stats[:, 4:8, :].sum(axis=(0, 1))

    cap = int(T / E * CAPACITY_FACTOR)
    if f_sum.max() > cap - 1.0:
        # Some expert's routed mass is near/over capacity: masking could
        # change the output.  Delegate to the exact host implementation.
        out, aux = _numpy_fallback(x, W, b)
        if _trace:
            return (out, aux), res
        return out, aux

    f = f_sum.astype(np.float32) / np.float32(T)
    P = p_sum.astype(np.float32) / np.float32(T)
    aux = np.float32(ALPHA * E) * np.float32((f * P).sum())
    out = probs.reshape(B, S, E)
    if _trace:
        return (out, aux), res
    return out, aux
